# revision 4
# baseline (speedup 1.0000x reference)
"""Multi-head attention (B=384, S=128, E=512, H=4, D=128) on 8 TRN2 NeuronCores.

Data-parallel: batch 384 -> 48 per core, projection weights replicated.

All-16-bit PE pipeline (PSUM accumulation stays fp32): x is converted to fp16
on the (otherwise idle) Pool engine, weights are converted to fp16 once at
startup, and every matmul/transpose runs with 16-bit operands so the PE does
1 cycle/row everywhere (fp32r matmuls with a 128 moving dim run at 2-4
cycles/row; fp32r transposes at 1.5).

Per-core dataflow:

  x16  = fp16(x)                 Pool tensor_copy
  xT   = transpose(x16)          PE transpose (fp16, 1 cyc/row), DVE eviction
  QT   = Wq16^T @ xT + bq        [E_out, rows] fp16, ACT eviction
  KT   = Wk16^T @ xT + bk        [E_out, rows] fp16, ACT eviction
  V    = x16 @ Wv16 + bv         [rows, E_out] bf16, DVE eviction
  per batch (4 heads packed along the PSUM free dim):
    S    = qT.T @ kT             [S, H, T] fp16 matmul -> fp32 PSUM
    w    = exp(S) bf16           ACT (no max-sub: |S| < 88 so bf16 exp cannot
                                 overflow; bf16 keeps the fp32 exponent range)
    sum  = reduce_sum(w)         DVE, recip on DVE
    wn   = w * (1/rowsum)        Pool (SBUF->SBUF), bf16
    wT   = transpose(wn)         PE transpose (bf16), DVE eviction
    attT = lhsT(v) @ wT          [D, H, S] bf16 matmul, ACT eviction -> fp16
  O    = att @ Wo16 + bo         (lhsT = attT chunk fp16, rhs = Wo16) -> f32

Engine budget per the measured baseline trace (280us, PE 97.6% busy):
PE ~205us (projections 163us is the 1-cyc/row floor for this op mix at
>=fp16 precision), ACT ~120us, DVE ~135us, Pool ~90us. Rel err ~3e-3
(gate 2e-2; validated in numpy: fp16 q/k path err ~5e-3 dominated by the
bf16 softmax weights, fp16 x/W quantization contributes ~4.5e-3).

Scheduling: engine streams execute in emission order; chunks are emitted as
a software pipeline -- scores(k) | projections(k+1) | x-transposes(k+2) |
attention-tail(k) -- so chunk k's softmax chain (ACT exp -> DVE sum/recip ->
Pool normalize) drains while the PE chews through chunk k+1's projections.
Short warmup (10 dummy bf16 matmuls) ramps the PE HAM clock-gate during the
initial x/weight DMA window; x batch-0 DMA is issued first so real PE work
can start as soon as it lands.
"""

import numpy as np

import concourse.bass as bass
import concourse.tile as tile
import concourse.mybir as mybir
from concourse import bacc
from concourse.bass_utils import run_bass_kernel_spmd
from concourse.masks import make_identity

B, S, E, H, D = 384, 128, 512, 4, 128
NCORES = 8
BLOC = B // NCORES  # 48 batches per core
NB = 4  # batches per chunk
NCHUNK = BLOC // NB
NBS = NB * S  # 512 rows of x per chunk
EC = E // 128  # 4 chunks of the embed dim

F32 = mybir.dt.float32
F32R = mybir.dt.float32r
BF16 = mybir.dt.bfloat16
F16 = mybir.dt.float16

_CACHE = {}


def build():
    nc = bacc.Bacc("TRN2", target_bir_lowering=False, debug=False, num_devices=NCORES)

    x = nc.dram_tensor("x", [BLOC, S, E], F32, kind="ExternalInput").ap()
    wq = nc.dram_tensor("Wq", [E, E], F32, kind="ExternalInput").ap()
    wk = nc.dram_tensor("Wk", [E, E], F32, kind="ExternalInput").ap()
    wv = nc.dram_tensor("Wv", [E, E], F32, kind="ExternalInput").ap()
    wo = nc.dram_tensor("Wo", [E, E], F32, kind="ExternalInput").ap()
    bq = nc.dram_tensor("bq", [E], F32, kind="ExternalInput").ap()
    bk = nc.dram_tensor("bk", [E], F32, kind="ExternalInput").ap()
    bv = nc.dram_tensor("bv", [E], F32, kind="ExternalInput").ap()
    bo = nc.dram_tensor("bo", [E], F32, kind="ExternalInput").ap()
    out = nc.dram_tensor("out", [BLOC, S, E], F32, kind="ExternalOutput").ap()

    with tile.TileContext(nc) as tc:
        with (
            tc.tile_pool(name="singles", bufs=1) as singles,
            tc.tile_pool(name="xp", bufs=2) as xp,
            tc.tile_pool(name="qkv", bufs=2) as qkv,
            tc.tile_pool(name="attn", bufs=2) as attn,
            tc.tile_pool(name="wsm", bufs=4) as wsm,
            tc.tile_pool(name="stats", bufs=8) as stats,
            tc.tile_pool(name="ps", bufs=8, space="PSUM") as ps,
        ):
            # --- weights / biases / identities ---
            w_stage = {}
            w16 = {}
            w_dram = {"q": wq, "k": wk, "v": wv, "o": wo}
            for name in ("q", "k", "v", "o"):
                w_stage[name] = singles.tile([128, EC, E], F32, tag=f"ws{name}", name=f"ws{name}")
                w16[name] = singles.tile([128, EC, E], F16, tag=f"w{name}", name=f"w{name}")

            def load_weight(name, conv_engine):
                for c in range(EC):
                    nc.sync.dma_start(
                        out=w_stage[name][:, c, :],
                        in_=w_dram[name][c * 128 : (c + 1) * 128, :],
                    )
                    if conv_engine is nc.scalar:
                        nc.scalar.copy(
                            out=w16[name][:, c, :], in_=w_stage[name][:, c, :]
                        )
                    else:
                        conv_engine.tensor_copy(
                            out=w16[name][:, c, :], in_=w_stage[name][:, c, :]
                        )

            bq_sb = singles.tile([128, EC], F32, tag="bq")
            bk_sb = singles.tile([128, EC], F32, tag="bk")
            bv_sb = singles.tile([128, E], F32, tag="bv")
            bo_sb = singles.tile([128, E], F32, tag="bo")

            def load_biases():
                for t, b in ((bq_sb, bq), (bk_sb, bk)):
                    nc.sync.dma_start(
                        out=t,
                        in_=bass.AP(tensor=b.tensor, offset=0, ap=[[1, 128], [128, EC]]),
                    )
                for t, b in ((bv_sb, bv), (bo_sb, bo)):
                    nc.sync.dma_start(
                        out=t,
                        in_=bass.AP(tensor=b.tensor, offset=0, ap=[[0, 128], [1, E]]),
                    )

            ident_f16 = singles.tile([128, 128], F16, tag="idf16")
            make_identity(nc, ident_f16[:])
            ident_bf = singles.tile([128, 128], BF16, tag="idb")
            make_identity(nc, ident_bf[:])

            # Warm the PE HAM clock-gate during the initial DMA window with a
            # few dummy matmuls (PE would otherwise start cold at half clock).
            dummy_bf = singles.tile([128, E], BF16, tag="dummy")
            nc.gpsimd.memset(dummy_bf, 0.0)
            warm_ps = ps.tile([128, E], F32, tag="ps", name="warm")
            for _ in range(10):
                nc.tensor.matmul(warm_ps, ident_bf[:], dummy_bf, start=True, stop=True)

            def emit_x_dma(chunk):
                b0 = chunk * NB
                x_nat = []
                for j in range(NB):
                    t = xp.tile([128, E], F32, tag=f"xnat{j}")
                    nc.sync.dma_start(out=t, in_=x[b0 + j])
                    x_nat.append(t)
                return x_nat

            def conv_trans(x_nat, by_batch=False):
                """fp16 convert (Pool) + PE transpose -> xT fp16 tiles.

                by_batch=True orders transposes j-major so the first chunk's
                PE work starts as soon as batch 0's DMA lands (startup path);
                steady-state chunks keep c-major order (one PSUM bank live)."""
                x16 = []
                for j in range(NB):
                    t = xp.tile([128, E], F16, tag=f"x16{j}")
                    nc.gpsimd.tensor_copy(out=t, in_=x_nat[j])
                    x16.append(t)
                xt = []
                if by_batch:
                    psts = [
                        ps.tile([128, NBS], F16, tag="ps", name=f"pst{c}")
                        for c in range(EC)
                    ]
                    for j in range(NB):
                        for c in range(EC):
                            nc.tensor.transpose(
                                psts[c][:, j * 128 : (j + 1) * 128],
                                x16[j][:, c * 128 : (c + 1) * 128],
                                ident_f16[:],
                            )
                    for c in range(EC):
                        t = xp.tile([128, NBS], F16, tag=f"xt{c}")
                        nc.vector.tensor_copy(out=t, in_=psts[c])
                        xt.append(t)
                    return xt
                for c in range(EC):
                    pst = ps.tile([128, NBS], F16, tag="ps")
                    for j in range(NB):
                        nc.tensor.transpose(
                            pst[:, j * 128 : (j + 1) * 128],
                            x16[j][:, c * 128 : (c + 1) * 128],
                            ident_f16[:],
                        )
                    t = xp.tile([128, NBS], F16, tag=f"xt{c}")
                    nc.vector.tensor_copy(out=t, in_=pst)
                    xt.append(t)
                return xt

            def load_trans(chunk):
                return conv_trans(emit_x_dma(chunk))

            def proj(xt):
                """QT/KT/V projections from xT (all-fp16 matmuls)."""
                qt, kt = [], []
                for h in range(H):
                    p = ps.tile([128, NBS], F32, tag="ps")
                    for c in range(EC):
                        nc.tensor.matmul(
                            p,
                            w16["q"][:, c, h * 128 : (h + 1) * 128],
                            xt[c],
                            start=(c == 0),
                            stop=(c == EC - 1),
                        )
                    t = qkv.tile([128, NBS], F16, tag=f"qt{h}")
                    nc.scalar.add(out=t, in_=p, add=bq_sb[:, h : h + 1])
                    qt.append(t)
                    p = ps.tile([128, NBS], F32, tag="ps")
                    for c in range(EC):
                        nc.tensor.matmul(
                            p,
                            w16["k"][:, c, h * 128 : (h + 1) * 128],
                            xt[c],
                            start=(c == 0),
                            stop=(c == EC - 1),
                        )
                    t = qkv.tile([128, NBS], F16, tag=f"kt{h}")
                    nc.scalar.add(out=t, in_=p, add=bk_sb[:, h : h + 1])
                    kt.append(t)
                v_sb = []
                for j in range(NB):
                    p = ps.tile([128, E], F32, tag="ps")
                    for c in range(EC):
                        nc.tensor.matmul(
                            p,
                            xt[c][:, j * 128 : (j + 1) * 128],
                            w16["v"][:, c, :],
                            start=(c == 0),
                            stop=(c == EC - 1),
                        )
                    t = qkv.tile([128, E], BF16, tag=f"v{j}")
                    nc.vector.tensor_add(out=t, in0=p, in1=bv_sb)
                    v_sb.append(t)
                return qt, kt, v_sb

            def attn_scores(qt, kt):
                """scores + softmax (no max-subtraction) -> normalized bf16 w."""
                wns = []
                for j in range(NB):
                    ps_s = ps.tile([128, H, 128], F32, tag="ps")
                    for h in range(H):
                        nc.tensor.matmul(
                            ps_s[:, h, :],
                            qt[h][:, j * 128 : (j + 1) * 128],
                            kt[h][:, j * 128 : (j + 1) * 128],
                            start=True,
                            stop=True,
                        )
                    w_exp = wsm.tile([128, H, 128], BF16, tag="wexp")
                    nc.scalar.activation(
                        out=w_exp,
                        in_=ps_s,
                        func=mybir.ActivationFunctionType.Exp,
                        bias=0.0,
                        scale=1.0,
                    )
                    sumexp = stats.tile([128, H], F32, tag="sumexp")
                    nc.vector.reduce_sum(
                        out=sumexp, in_=w_exp, axis=mybir.AxisListType.X
                    )
                    recip = stats.tile([128, H], F32, tag="recip")
                    nc.vector.reciprocal(out=recip, in_=sumexp)
                    wn = wsm.tile([128, H, 128], BF16, tag="wn")
                    for h in range(H):
                        nc.gpsimd.tensor_scalar_mul(
                            out=wn[:, h, :],
                            in0=w_exp[:, h, :],
                            scalar1=recip[:, h : h + 1],
                        )
                    wns.append(wn)
                return wns

            def attn_tail(chunk, wns, v_sb):
                """wT transposes, att = v.T-form matmuls, O projection, store."""
                b0 = chunk * NB
                wt_sbs = []
                for j in range(NB):
                    ps_wt = ps.tile([128, H, 128], BF16, tag="ps")
                    for h in range(H):
                        nc.tensor.transpose(ps_wt[:, h, :], wns[j][:, h, :], ident_bf[:])
                    wt_sb = wsm.tile([128, H, 128], BF16, tag=f"wt{j}")
                    nc.vector.tensor_copy(out=wt_sb, in_=ps_wt)
                    wt_sbs.append(wt_sb)
                ats = []
                for j in range(NB):
                    ps_at = ps.tile([128, H, 128], F32, tag="ps")
                    for h in range(H):
                        nc.tensor.matmul(
                            ps_at[:, h, :],
                            v_sb[j][:, h * 128 : (h + 1) * 128],
                            wt_sbs[j][:, h, :],
                            start=True,
                            stop=True,
                        )
                    at = attn.tile([128, H, 128], F16, tag=f"at{j}")
                    nc.scalar.copy(out=at, in_=ps_at)
                    ats.append(at)
                for j in range(NB):
                    p = ps.tile([128, E], F32, tag="ps")
                    for h in range(H):
                        nc.tensor.matmul(
                            p,
                            ats[j][:, h, :],
                            w16["o"][:, h, :],
                            start=(h == 0),
                            stop=(h == H - 1),
                        )
                    o_sb = attn.tile([128, E], F32, tag=f"o{j}")
                    nc.vector.tensor_add(out=o_sb, in0=p, in1=bo_sb)
                    nc.sync.dma_start(out=out[b0 + j], in_=o_sb)

            # Software pipeline. Per iteration the PE stream is:
            #   scores(k) | projections(k+1) | transposes(k+2) | tail(k)
            # (transposes sit after the projections so the Pool fp16 converts
            # of chunk k+2 have a full projection's worth of time to land).
            x0 = emit_x_dma(0)
            load_biases()
            xts = {0: conv_trans(x0, by_batch=True)}
            load_weight("q", nc.vector)
            load_weight("k", nc.scalar)
            load_weight("v", nc.gpsimd)
            load_weight("o", nc.gpsimd)
            states = {0: proj(xts[0])}
            xts[1] = load_trans(1) if NCHUNK > 1 else None
            for k in range(NCHUNK):
                wns = attn_scores(states[k][0], states[k][1])
                if k + 1 < NCHUNK:
                    states[k + 1] = proj(xts[k + 1])
                if k + 2 < NCHUNK:
                    xts[k + 2] = load_trans(k + 2)
                attn_tail(k, wns, states[k][2])

    nc.compile()
    return nc


def kernel(**inputs):
    if "nc" not in _CACHE:
        _CACHE["nc"] = build()
    nc = _CACHE["nc"]

    x = np.ascontiguousarray(np.asarray(inputs["x"], dtype=np.float32))
    shared = {
        k: np.ascontiguousarray(np.asarray(inputs[k], dtype=np.float32))
        for k in ("Wq", "Wk", "Wv", "Wo", "bq", "bk", "bv", "bo")
    }
    in_maps = [
        {"x": x[i * BLOC : (i + 1) * BLOC], **shared} for i in range(NCORES)
    ]
    res = run_bass_kernel_spmd(nc, in_maps, core_ids=list(range(NCORES)))
    return np.concatenate([res.results[i]["out"] for i in range(NCORES)], axis=0)


# revision 5
# speedup vs baseline: 2.6396x; 2.6396x over previous
"""Multi-head attention (B=384, S=128, E=512, H=4, D=128) on 8 TRN2 NeuronCores.

Data-parallel: batch 384 -> 48 per core, projection weights replicated.

All-16-bit PE pipeline (PSUM accumulation stays fp32): x is converted to fp16
on the (otherwise idle) Pool engine, weights are converted to fp16 once at
startup, and every matmul/transpose runs with 16-bit operands so the PE does
1 cycle/row everywhere (fp32r matmuls with a 128 moving dim run at 2-4
cycles/row; fp32r transposes at 1.5).

Per-core dataflow:

  x16  = fp16(x)                 Pool tensor_copy
  xT   = transpose(x16)          PE transpose (fp16, 1 cyc/row), DVE eviction
  QT   = Wq16^T @ xT + bq        [E_out, rows] fp16, ACT eviction
  KT   = Wk16^T @ xT + bk        [E_out, rows] fp16, ACT eviction
  V    = x16 @ Wv16 + bv         [rows, E_out] bf16, DVE eviction
  per batch (4 heads packed along the PSUM free dim):
    S    = qT.T @ kT             [S, H, T] fp16 matmul -> fp32 PSUM
    w    = exp(S) bf16           ACT (no max-sub: |S| < 88 so bf16 exp cannot
                                 overflow; bf16 keeps the fp32 exponent range)
    sum  = reduce_sum(w)         DVE, recip on DVE
    wn   = w * (1/rowsum)        Pool (SBUF->SBUF), bf16
    wT   = transpose(wn)         PE transpose (bf16), DVE eviction
    attT = lhsT(v) @ wT          [D, H, S] bf16 matmul, ACT eviction -> fp16
  O    = att @ Wo16 + bo         (lhsT = attT chunk fp16, rhs = Wo16) -> f32

Engine budget per the measured baseline trace (280us, PE 97.6% busy):
PE ~205us (projections 163us is the 1-cyc/row floor for this op mix at
>=fp16 precision), ACT ~120us, DVE ~135us, Pool ~90us. Rel err ~3e-3
(gate 2e-2; validated in numpy: fp16 q/k path err ~5e-3 dominated by the
bf16 softmax weights, fp16 x/W quantization contributes ~4.5e-3).

Scheduling: engine streams execute in emission order; chunks are emitted as
a software pipeline -- scores(k) | projections(k+1) | x-transposes(k+2) |
attention-tail(k) -- so chunk k's softmax chain (ACT exp -> DVE sum/recip ->
Pool normalize) drains while the PE chews through chunk k+1's projections.
Short warmup (10 dummy bf16 matmuls) ramps the PE HAM clock-gate during the
initial x/weight DMA window; x batch-0 DMA is issued first so real PE work
can start as soon as it lands.
"""

import numpy as np

import concourse.bass as bass
import concourse.tile as tile
import concourse.mybir as mybir
from concourse import bacc
from concourse.bass_utils import run_bass_kernel_spmd
from concourse.masks import make_identity

B, S, E, H, D = 384, 128, 512, 4, 128
NCORES = 8
BLOC = B // NCORES  # 48 batches per core
NB = 4  # batches per chunk
NCHUNK = BLOC // NB
NBS = NB * S  # 512 rows of x per chunk
EC = E // 128  # 4 chunks of the embed dim

F32 = mybir.dt.float32
F32R = mybir.dt.float32r
BF16 = mybir.dt.bfloat16
F16 = mybir.dt.float16

_CACHE = {}


def build():
    nc = bacc.Bacc("TRN2", target_bir_lowering=False, debug=False, num_devices=NCORES)

    x = nc.dram_tensor("x", [BLOC, S, E], F32R, kind="ExternalInput").ap()
    wq = nc.dram_tensor("Wq", [E, E], F32, kind="ExternalInput").ap()
    wk = nc.dram_tensor("Wk", [E, E], F32, kind="ExternalInput").ap()
    wv = nc.dram_tensor("Wv", [E, E], F32, kind="ExternalInput").ap()
    wo = nc.dram_tensor("Wo", [E, E], F32, kind="ExternalInput").ap()
    bq = nc.dram_tensor("bq", [E], F32, kind="ExternalInput").ap()
    bk = nc.dram_tensor("bk", [E], F32, kind="ExternalInput").ap()
    bv = nc.dram_tensor("bv", [E], F32, kind="ExternalInput").ap()
    bo = nc.dram_tensor("bo", [E], F32, kind="ExternalInput").ap()
    out = nc.dram_tensor("out", [BLOC, S, E], F32, kind="ExternalOutput").ap()

    with tile.TileContext(nc) as tc:
        with (
            tc.tile_pool(name="singles", bufs=1) as singles,
            tc.tile_pool(name="xp", bufs=2) as xp,
            tc.tile_pool(name="qkv", bufs=2) as qkv,
            tc.tile_pool(name="attn", bufs=2) as attn,
            tc.tile_pool(name="wsm", bufs=4) as wsm,
            tc.tile_pool(name="stats", bufs=8) as stats,
            tc.tile_pool(name="ps", bufs=8, space="PSUM") as ps,
        ):
            # --- weights / biases / identities ---
            w_stage = {}
            w16 = {}
            w_dram = {"q": wq, "k": wk, "v": wv, "o": wo}
            for name in ("q", "k", "v", "o"):
                w_stage[name] = singles.tile([128, EC, E], F32, tag=f"ws{name}", name=f"ws{name}")
                w16[name] = singles.tile([128, EC, E], F16, tag=f"w{name}", name=f"w{name}")

            def load_weight(name, conv_engine):
                for c in range(EC):
                    nc.sync.dma_start(
                        out=w_stage[name][:, c, :],
                        in_=w_dram[name][c * 128 : (c + 1) * 128, :],
                    )
                    if conv_engine is nc.scalar:
                        nc.scalar.copy(
                            out=w16[name][:, c, :], in_=w_stage[name][:, c, :]
                        )
                    else:
                        conv_engine.tensor_copy(
                            out=w16[name][:, c, :], in_=w_stage[name][:, c, :]
                        )

            bq_sb = singles.tile([128, EC], F32, tag="bq")
            bk_sb = singles.tile([128, EC], F32, tag="bk")
            bv_sb = singles.tile([128, E], F32, tag="bv")
            bo_sb = singles.tile([128, E], F32, tag="bo")

            def load_biases():
                for t, b in ((bq_sb, bq), (bk_sb, bk)):
                    nc.sync.dma_start(
                        out=t,
                        in_=bass.AP(tensor=b.tensor, offset=0, ap=[[1, 128], [128, EC]]),
                    )
                for t, b in ((bv_sb, bv), (bo_sb, bo)):
                    nc.sync.dma_start(
                        out=t,
                        in_=bass.AP(tensor=b.tensor, offset=0, ap=[[0, 128], [1, E]]),
                    )

            ident_f32 = singles.tile([128, 128], F32, tag="idf32")
            make_identity(nc, ident_f32[:])
            ident = singles.tile([128, 128], F32R, tag="idf")
            nc.vector.tensor_copy(out=ident, in_=ident_f32[:].bitcast(F32R))
            ident_bf = singles.tile([128, 128], BF16, tag="idb")
            make_identity(nc, ident_bf[:])

            # Warm the PE HAM clock-gate during the initial DMA window with a
            # few dummy matmuls (PE would otherwise start cold at half clock).
            dummy_bf = singles.tile([128, E], BF16, tag="dummy")
            nc.vector.memset(dummy_bf, 0.0)
            warm_ps = ps.tile([128, E], F32, tag="ps", name="warm")
            for _ in range(10):
                nc.tensor.matmul(warm_ps, ident_bf[:], dummy_bf, start=True, stop=True)

            def emit_x_dma(chunk):
                b0 = chunk * NB
                x_nat = []
                for j in range(NB):
                    t = xp.tile([128, E], F32R, tag=f"xnat{j}")
                    nc.sync.dma_start(out=t, in_=x[b0 + j])
                    x_nat.append(t)
                return x_nat

            def conv_trans(x_nat, by_batch=False):
                """PE transpose (f32r) -> xT fp16 tiles (ACT eviction casts).

                by_batch=True orders transposes j-major so the first chunk's
                PE work starts as soon as batch 0's DMA lands (startup path);
                steady-state chunks keep c-major order (one PSUM bank live)."""
                xt = []
                if by_batch:
                    psts = [
                        ps.tile([128, NBS], F32R, tag="ps", name=f"pst{c}")
                        for c in range(EC)
                    ]
                    for j in range(NB):
                        for c in range(EC):
                            nc.tensor.transpose(
                                psts[c][:, j * 128 : (j + 1) * 128],
                                x_nat[j][:, c * 128 : (c + 1) * 128],
                                ident[:],
                            )
                    for c in range(EC):
                        t = xp.tile([128, NBS], F16, tag=f"xt{c}")
                        nc.scalar.copy(out=t, in_=psts[c].bitcast(F32))
                        xt.append(t)
                    return xt
                for c in range(EC):
                    pst = ps.tile([128, NBS], F32R, tag="ps")
                    for j in range(NB):
                        nc.tensor.transpose(
                            pst[:, j * 128 : (j + 1) * 128],
                            x_nat[j][:, c * 128 : (c + 1) * 128],
                            ident[:],
                        )
                    t = xp.tile([128, NBS], F16, tag=f"xt{c}")
                    nc.scalar.copy(out=t, in_=pst.bitcast(F32))
                    xt.append(t)
                return xt

            def load_trans(chunk):
                return conv_trans(emit_x_dma(chunk))

            def proj(xt):
                """QT/KT/V projections from xT (all-fp16 matmuls)."""
                qt, kt = [], []
                for h in range(H):
                    p = ps.tile([128, NBS], F32, tag="ps")
                    for c in range(EC):
                        nc.tensor.matmul(
                            p,
                            w16["q"][:, c, h * 128 : (h + 1) * 128],
                            xt[c],
                            start=(c == 0),
                            stop=(c == EC - 1),
                        )
                    t = qkv.tile([128, NBS], F16, tag=f"qt{h}")
                    nc.scalar.add(out=t, in_=p, add=bq_sb[:, h : h + 1])
                    qt.append(t)
                    p = ps.tile([128, NBS], F32, tag="ps")
                    for c in range(EC):
                        nc.tensor.matmul(
                            p,
                            w16["k"][:, c, h * 128 : (h + 1) * 128],
                            xt[c],
                            start=(c == 0),
                            stop=(c == EC - 1),
                        )
                    t = qkv.tile([128, NBS], F16, tag=f"kt{h}")
                    nc.scalar.add(out=t, in_=p, add=bk_sb[:, h : h + 1])
                    kt.append(t)
                v_sb = []
                for j in range(NB):
                    p = ps.tile([128, E], F32, tag="ps")
                    for c in range(EC):
                        nc.tensor.matmul(
                            p,
                            xt[c][:, j * 128 : (j + 1) * 128],
                            w16["v"][:, c, :],
                            start=(c == 0),
                            stop=(c == EC - 1),
                        )
                    t = qkv.tile([128, E], BF16, tag=f"v{j}")
                    nc.vector.tensor_add(out=t, in0=p, in1=bv_sb)
                    v_sb.append(t)
                return qt, kt, v_sb

            def attn_scores(qt, kt):
                """scores + softmax (no max-subtraction) -> normalized bf16 w."""
                wns = []
                for j in range(NB):
                    ps_s = ps.tile([128, H, 128], F32, tag="ps")
                    for h in range(H):
                        nc.tensor.matmul(
                            ps_s[:, h, :],
                            qt[h][:, j * 128 : (j + 1) * 128],
                            kt[h][:, j * 128 : (j + 1) * 128],
                            start=True,
                            stop=True,
                        )
                    w_exp = wsm.tile([128, H, 128], BF16, tag="wexp")
                    nc.scalar.activation(
                        out=w_exp,
                        in_=ps_s,
                        func=mybir.ActivationFunctionType.Exp,
                        bias=0.0,
                        scale=1.0,
                    )
                    sumexp = stats.tile([128, H], F32, tag="sumexp")
                    nc.vector.reduce_sum(
                        out=sumexp, in_=w_exp, axis=mybir.AxisListType.X
                    )
                    recip = stats.tile([128, H], F32, tag="recip")
                    nc.vector.reciprocal(out=recip, in_=sumexp)
                    wn = wsm.tile([128, H, 128], BF16, tag="wn")
                    for h in range(H):
                        nc.vector.tensor_scalar_mul(
                            out=wn[:, h, :],
                            in0=w_exp[:, h, :],
                            scalar1=recip[:, h : h + 1],
                        )
                    wns.append(wn)
                return wns

            def attn_tail(chunk, wns, v_sb):
                """wT transposes, att = v.T-form matmuls, O projection, store."""
                b0 = chunk * NB
                wt_sbs = []
                for j in range(NB):
                    ps_wt = ps.tile([128, H, 128], BF16, tag="ps")
                    for h in range(H):
                        nc.tensor.transpose(ps_wt[:, h, :], wns[j][:, h, :], ident_bf[:])
                    wt_sb = wsm.tile([128, H, 128], BF16, tag=f"wt{j}")
                    nc.vector.tensor_copy(out=wt_sb, in_=ps_wt)
                    wt_sbs.append(wt_sb)
                ats = []
                for j in range(NB):
                    ps_at = ps.tile([128, H, 128], F32, tag="ps")
                    for h in range(H):
                        nc.tensor.matmul(
                            ps_at[:, h, :],
                            v_sb[j][:, h * 128 : (h + 1) * 128],
                            wt_sbs[j][:, h, :],
                            start=True,
                            stop=True,
                        )
                    at = attn.tile([128, H, 128], F16, tag=f"at{j}")
                    nc.scalar.copy(out=at, in_=ps_at)
                    ats.append(at)
                for j in range(NB):
                    p = ps.tile([128, E], F32, tag="ps")
                    for h in range(H):
                        nc.tensor.matmul(
                            p,
                            ats[j][:, h, :],
                            w16["o"][:, h, :],
                            start=(h == 0),
                            stop=(h == H - 1),
                        )
                    o_sb = attn.tile([128, E], F32, tag=f"o{j}")
                    nc.vector.tensor_add(out=o_sb, in0=p, in1=bo_sb)
                    nc.sync.dma_start(out=out[b0 + j], in_=o_sb)

            # Software pipeline. Per iteration the PE stream is:
            #   scores(k) | projections(k+1) | transposes(k+2) | tail(k)
            # (transposes sit after the projections so the Pool fp16 converts
            # of chunk k+2 have a full projection's worth of time to land).
            x0 = emit_x_dma(0)
            load_biases()
            xts = {0: conv_trans(x0, by_batch=True)}
            load_weight("q", nc.vector)
            load_weight("k", nc.vector)
            load_weight("v", nc.scalar)
            load_weight("o", nc.scalar)
            states = {0: proj(xts[0])}
            xts[1] = load_trans(1) if NCHUNK > 1 else None
            for k in range(NCHUNK):
                wns = attn_scores(states[k][0], states[k][1])
                if k + 1 < NCHUNK:
                    states[k + 1] = proj(xts[k + 1])
                if k + 2 < NCHUNK:
                    xts[k + 2] = load_trans(k + 2)
                attn_tail(k, wns, states[k][2])

    nc.compile()
    return nc


def kernel(**inputs):
    if "nc" not in _CACHE:
        _CACHE["nc"] = build()
    nc = _CACHE["nc"]

    x = np.ascontiguousarray(np.asarray(inputs["x"], dtype=np.float32))
    shared = {
        k: np.ascontiguousarray(np.asarray(inputs[k], dtype=np.float32))
        for k in ("Wq", "Wk", "Wv", "Wo", "bq", "bk", "bv", "bo")
    }
    in_maps = [
        {"x": x[i * BLOC : (i + 1) * BLOC], **shared} for i in range(NCORES)
    ]
    res = run_bass_kernel_spmd(nc, in_maps, core_ids=list(range(NCORES)))
    return np.concatenate([res.results[i]["out"] for i in range(NCORES)], axis=0)


# revision 15
# speedup vs baseline: 2.7450x; 1.0399x over previous
"""Multi-head attention (B=384, S=128, E=512, H=4, D=128) on 8 TRN2 NeuronCores.

Data-parallel: batch 384 -> 48 per core, projection weights replicated.

All-16-bit PE pipeline (PSUM accumulation stays fp32): weights are
pre-converted to fp16 on the host (halves the startup weight DMA and kills
all on-chip conversion work), x is converted to fp16 on ACT/DVE before its
PE transpose, and the PSUM evictions cast everything to fp16/bf16 so every
matmul and transpose runs 16-bit at 1 cycle/row (fp32r matmuls with a 128
moving dim run at 2 cycles/row on HW -- ~107ns vs ~53ns measured for the
score matmuls -- and fp32r transposes at 1.5).

Per-core dataflow:

  x16  = fp16(x)                 ACT/DVE halves (GpSimd is useless: ~2.4us
                                 fixed per-op software cost measured)
  xT   = transpose(x16)          PE transpose (fp16), ACT eviction
  QT   = Wq16^T @ xT + bq        [E_out, rows], ACT eviction -> fp16
  KT   = Wk16^T @ xT + bk        [E_out, rows], ACT eviction -> fp16
  V    = xT-chunks @ Wv16 + bv   [rows, E_out], DVE eviction -> bf16
  per batch (4 heads packed along the PSUM free dim):
    S    = qT.T @ kT             [S, H, T] fp16 matmul -> fp32 PSUM
    w    = exp(S) bf16           ACT (no max-sub: |S| < 88 so bf16 exp cannot
                                 overflow; bf16 keeps the fp32 exponent range)
    sum  = reduce_sum(w)         DVE, reciprocal DVE
    wn   = w * (1/rowsum)        DVE (2-byte 2x mode), bf16
    wT   = transpose(wn)         PE transpose (bf16), DVE eviction
    attT = lhsT(v) @ wT          [D, H, S] bf16 matmul, ACT eviction -> fp16
  O    = att @ Wo16 + bo         (lhsT = attT chunk fp16, rhs = Wo16) -> f32

Engine budget (measured warm): PE ~222us busy (projections ~164us = the
1-cyc/row floor for this op mix at >=fp16 precision; 768 small matmuls
~50us at their ~53ns floor), ACT ~154us, DVE ~160us, GpSimd idle.
Rel err 2.8e-3 (gate 2e-2): fp16 x/W quantization in the q/k path gives
~5e-3 worst case, bf16 softmax weights ~3e-3 (validated in numpy).
Run-to-run HW variance is +/-10-20% (chip-wide clock throttle state).

Startup: every DMA dispatch costs ~650ns-2us serially on its issuing
sequencer (descriptor generation), so x loads go on the SP queue while the
(single-instruction-per-matrix, host-packed) weight + bias loads go on the
ACT queue in parallel, and chunk 0/1 conversions+evictions run on the DVE
(the ACT queue is still generating weight descriptors then). ~6 dummy bf16
matmuls ramp the PE HAM clock-gate while the first x tiles land. Steady
state emits scores(k) | projections(k+1) | x-transposes(k+2) | tail(k) so
chunk k's softmax chain drains on ACT/DVE under chunk k+1's projections;
the last two chunks interleave their tails to keep the PE fed through the
drain (tail-O of chunk 10 fills the softmax latency of chunk 11), and the
final stores split across both hwdge queues per batch.
"""

import numpy as np

import concourse.bass as bass
import concourse.tile as tile
import concourse.mybir as mybir
from concourse import bacc
from concourse.bass_utils import run_bass_kernel_spmd
from concourse.masks import make_identity

B, S, E, H, D = 384, 128, 512, 4, 128
NCORES = 8
BLOC = B // NCORES  # 48 batches per core
NB = 4  # batches per chunk
NCHUNK = BLOC // NB
NBS = NB * S  # 512 rows of x per chunk
EC = E // 128  # 4 chunks of the embed dim

F32 = mybir.dt.float32
F32R = mybir.dt.float32r
BF16 = mybir.dt.bfloat16
F16 = mybir.dt.float16

_CACHE = {}


def build():
    nc = bacc.Bacc("TRN2", target_bir_lowering=False, debug=False, num_devices=NCORES)

    x = nc.dram_tensor("x", [BLOC, S, E], F32R, kind="ExternalInput").ap()
    wq16 = nc.dram_tensor("Wq16", [E, E], F16, kind="ExternalInput").ap()
    wk16 = nc.dram_tensor("Wk16", [E, E], F16, kind="ExternalInput").ap()
    wv16 = nc.dram_tensor("Wv16", [E, E], F16, kind="ExternalInput").ap()
    wo16 = nc.dram_tensor("Wo16", [E, E], F16, kind="ExternalInput").ap()
    bqp = nc.dram_tensor("bqp", [128, EC], F32, kind="ExternalInput").ap()
    bkp = nc.dram_tensor("bkp", [128, EC], F32, kind="ExternalInput").ap()
    bv = nc.dram_tensor("bv", [E], F32, kind="ExternalInput").ap()
    bo = nc.dram_tensor("bo", [E], F32, kind="ExternalInput").ap()
    out = nc.dram_tensor("out", [BLOC, S, E], F32, kind="ExternalOutput").ap()

    with tile.TileContext(nc) as tc:
        with (
            tc.tile_pool(name="singles", bufs=1) as singles,
            tc.tile_pool(name="xp", bufs=2) as xp,
            tc.tile_pool(name="qkv", bufs=2) as qkv,
            tc.tile_pool(name="attn", bufs=2) as attn,
            tc.tile_pool(name="wsm", bufs=4) as wsm,
            tc.tile_pool(name="stats", bufs=8) as stats,
            tc.tile_pool(name="ps", bufs=8, space="PSUM") as ps,
        ):
            w16 = {}
            w_dram = {"q": wq16, "k": wk16, "v": wv16, "o": wo16}
            for name in ("q", "k", "v", "o"):
                w16[name] = singles.tile(
                    [128, EC, E], F16, tag=f"w{name}", name=f"w{name}"
                )

            def load_weights():
                # One dispatch per matrix, on the ACT hwdge queue so the SP
                # queue is free for the x loads.
                for name in ("q", "k", "v", "o"):
                    nc.scalar.dma_start(
                        out=w16[name][:, :, :],
                        in_=bass.AP(
                            tensor=w_dram[name].tensor,
                            offset=0,
                            ap=[[E, 128], [128 * E, EC], [1, E]],
                        ),
                    )

            bq_sb = singles.tile([128, EC], F32, tag="bq")
            bk_sb = singles.tile([128, EC], F32, tag="bk")
            bv_sb = singles.tile([128, E], F32, tag="bv")
            bo_sb = singles.tile([128, E], F32, tag="bo")

            def load_biases():
                nc.scalar.dma_start(out=bq_sb, in_=bqp[:, :])
                nc.scalar.dma_start(out=bk_sb, in_=bkp[:, :])
                for t, b in ((bv_sb, bv), (bo_sb, bo)):
                    nc.scalar.dma_start(
                        out=t,
                        in_=bass.AP(tensor=b.tensor, offset=0, ap=[[0, 128], [1, E]]),
                    )

            ident_f16 = singles.tile([128, 128], F16, tag="idf16")
            make_identity(nc, ident_f16[:])
            ident_bf = singles.tile([128, 128], BF16, tag="idb")
            make_identity(nc, ident_bf[:])

            # Warm the PE HAM clock-gate while the first x tiles land (PE
            # would otherwise start cold at half clock).
            dummy_bf = singles.tile([128, E], BF16, tag="dummy")
            nc.vector.memset(dummy_bf, 0.0)
            warm_ps = ps.tile([128, E], F32, tag="ps", name="warm")
            for _ in range(6):
                nc.tensor.matmul(warm_ps, ident_bf[:], dummy_bf, start=True, stop=True)

            def emit_x_dma(chunk, fine=False):
                b0 = chunk * NB
                t = xp.tile([128, NB, E], F32R, tag="xnat")
                if fine:
                    # separate dispatches overlap descriptor generation with
                    # the transfer, getting the first bytes moving sooner
                    for j in range(NB):
                        nc.sync.dma_start(out=t[:, j, :], in_=x[b0 + j])
                else:
                    nc.sync.dma_start(
                        out=t,
                        in_=bass.AP(
                            tensor=x.tensor,
                            offset=b0 * S * E,
                            ap=[[E, 128], [S * E, NB], [1, E]],
                        ),
                    )
                return t

            def conv_trans0(x_nat):
                """Startup path for chunk 0: per-batch DVE converts + j-major
                transposes so PE work starts as soon as batch 0 lands."""
                x16 = xp.tile([128, NB, E], F16, tag="x16")
                for j in range(NB):
                    nc.vector.tensor_copy(
                        out=x16[:, j, :], in_=x_nat[:, j, :].bitcast(F32)
                    )
                psts = [
                    ps.tile([128, NBS], F16, tag="ps", name=f"pst{c}")
                    for c in range(EC)
                ]
                for j in range(NB):
                    for c in range(EC):
                        nc.tensor.transpose(
                            psts[c][:, j * 128 : (j + 1) * 128],
                            x16[:, j, c * 128 : (c + 1) * 128],
                            ident_f16[:],
                        )
                xt = []
                for c in range(EC):
                    t = xp.tile([128, NBS], F16, tag=f"xt{c}")
                    nc.vector.tensor_copy(out=t, in_=psts[c])
                    xt.append(t)
                return xt

            def conv_trans(x_nat, dve_only=False):
                """fp16 convert (ACT+DVE halves) + PE transpose -> xT fp16.

                dve_only routes the converts and evictions to the DVE for the
                prelude chunks: at startup the ACT hwdge queue is busy
                generating weight/bias DMA descriptors and would stall the
                first transposes by ~10us."""
                x16 = xp.tile([128, NB, E], F16, tag="x16")
                if dve_only:
                    nc.vector.tensor_copy(out=x16, in_=x_nat[:, :, :].bitcast(F32))
                else:
                    nc.scalar.copy(
                        out=x16[:, 0:2, :], in_=x_nat[:, 0:2, :].bitcast(F32)
                    )
                    nc.vector.tensor_copy(
                        out=x16[:, 2:4, :], in_=x_nat[:, 2:4, :].bitcast(F32)
                    )
                xt = []
                for c in range(EC):
                    pst = ps.tile([128, NBS], F16, tag="ps")
                    for j in range(NB):
                        nc.tensor.transpose(
                            pst[:, j * 128 : (j + 1) * 128],
                            x16[:, j, c * 128 : (c + 1) * 128],
                            ident_f16[:],
                        )
                    t = xp.tile([128, NBS], F16, tag=f"xt{c}")
                    if dve_only:
                        nc.vector.tensor_copy(out=t, in_=pst)
                    else:
                        nc.scalar.copy(out=t, in_=pst)
                    xt.append(t)
                return xt

            def load_trans(chunk):
                return conv_trans(emit_x_dma(chunk))

            def proj(xt):
                """QT/KT/V projections from xT (fp16 operands, fp32 PSUM)."""
                qt, kt = [], []
                for h in range(H):
                    p = ps.tile([128, NBS], F32, tag="ps")
                    for c in range(EC):
                        nc.tensor.matmul(
                            p,
                            w16["q"][:, c, h * 128 : (h + 1) * 128],
                            xt[c],
                            start=(c == 0),
                            stop=(c == EC - 1),
                        )
                    t = qkv.tile([128, NBS], F16, tag=f"qt{h}")
                    nc.scalar.add(out=t, in_=p, add=bq_sb[:, h : h + 1])
                    qt.append(t)
                    p = ps.tile([128, NBS], F32, tag="ps")
                    for c in range(EC):
                        nc.tensor.matmul(
                            p,
                            w16["k"][:, c, h * 128 : (h + 1) * 128],
                            xt[c],
                            start=(c == 0),
                            stop=(c == EC - 1),
                        )
                    t = qkv.tile([128, NBS], F16, tag=f"kt{h}")
                    nc.scalar.add(out=t, in_=p, add=bk_sb[:, h : h + 1])
                    kt.append(t)
                v_sb = []
                for j in range(NB):
                    p = ps.tile([128, E], F32, tag="ps")
                    for c in range(EC):
                        nc.tensor.matmul(
                            p,
                            xt[c][:, j * 128 : (j + 1) * 128],
                            w16["v"][:, c, :],
                            start=(c == 0),
                            stop=(c == EC - 1),
                        )
                    t = qkv.tile([128, E], BF16, tag=f"v{j}")
                    nc.vector.tensor_add(out=t, in0=p, in1=bv_sb)
                    v_sb.append(t)
                return qt, kt, v_sb

            def attn_scores(qt, kt, js=None, act_norms=False):
                """scores + softmax (no max-subtraction) -> normalized bf16 w."""
                wns = []
                for j in (range(NB) if js is None else js):
                    ps_s = ps.tile([128, H, 128], F32, tag="ps")
                    for h in range(H):
                        nc.tensor.matmul(
                            ps_s[:, h, :],
                            qt[h][:, j * 128 : (j + 1) * 128],
                            kt[h][:, j * 128 : (j + 1) * 128],
                            start=True,
                            stop=True,
                        )
                    w_exp = wsm.tile([128, H, 128], BF16, tag="wexp")
                    nc.scalar.activation(
                        out=w_exp,
                        in_=ps_s,
                        func=mybir.ActivationFunctionType.Exp,
                        bias=0.0,
                        scale=1.0,
                    )
                    sumexp = stats.tile([128, H], F32, tag="sumexp")
                    nc.vector.reduce_sum(
                        out=sumexp, in_=w_exp, axis=mybir.AxisListType.X
                    )
                    recip = stats.tile([128, H], F32, tag="recip")
                    nc.vector.reciprocal(out=recip, in_=sumexp)
                    wn = wsm.tile([128, H, 128], BF16, tag="wn")
                    for h in range(H):
                        if act_norms and h < 2:
                            nc.scalar.activation(
                                out=wn[:, h, :],
                                in_=w_exp[:, h, :],
                                func=mybir.ActivationFunctionType.Identity,
                                bias=0.0,
                                scale=recip[:, h : h + 1],
                            )
                        else:
                            nc.vector.tensor_scalar_mul(
                                out=wn[:, h, :],
                                in0=w_exp[:, h, :],
                                scalar1=recip[:, h : h + 1],
                            )
                    wns.append(wn)
                return wns

            def tail_watt(chunk, wns, v_sb, js=None):
                """wT transposes + att = v.T-form matmuls -> at fp16 tiles."""
                js = range(NB) if js is None else js
                wn_of = wns if callable(wns) else lambda j: wns[j]
                wt_sbs = {}
                for j in js:
                    ps_wt = ps.tile([128, H, 128], BF16, tag="ps")
                    for h in range(H):
                        nc.tensor.transpose(
                            ps_wt[:, h, :], wn_of(j)[:, h, :], ident_bf[:]
                        )
                    wt_sb = wsm.tile([128, H, 128], BF16, tag=f"wt{j}")
                    nc.vector.tensor_copy(out=wt_sb, in_=ps_wt)
                    wt_sbs[j] = wt_sb
                ats = {}
                for j in js:
                    ps_at = ps.tile([128, H, 128], F32, tag="ps")
                    for h in range(H):
                        nc.tensor.matmul(
                            ps_at[:, h, :],
                            v_sb[j][:, h * 128 : (h + 1) * 128],
                            wt_sbs[j][:, h, :],
                            start=True,
                            stop=True,
                        )
                    at = attn.tile([128, H, 128], F16, tag=f"at{j}")
                    nc.scalar.copy(out=at, in_=ps_at)
                    ats[j] = at
                return ats

            def tail_o(chunk, ats, js=None, o_sb=None):
                """O projection + bias + store."""
                b0 = chunk * NB
                js = range(NB) if js is None else js
                split_store = chunk == NCHUNK - 1
                if o_sb is None:
                    o_sb = attn.tile([128, NB, E], F32, tag="o")
                for j in js:
                    p = ps.tile([128, E], F32, tag="ps")
                    for h in range(H):
                        nc.tensor.matmul(
                            p,
                            ats[j][:, h, :],
                            w16["o"][:, h, :],
                            start=(h == 0),
                            stop=(h == H - 1),
                        )
                    nc.vector.tensor_add(out=o_sb[:, j, :], in0=p, in1=bo_sb)
                    if split_store:
                        eng = nc.sync if j % 2 == 0 else nc.scalar
                        eng.dma_start(out=out[b0 + j], in_=o_sb[:, j, :])
                if not split_store and (NB - 1) in js:
                    nc.sync.dma_start(
                        out=bass.AP(
                            tensor=out.tensor,
                            offset=b0 * S * E,
                            ap=[[E, 128], [S * E, NB], [1, E]],
                        ),
                        in_=o_sb,
                    )
                return o_sb

            # Startup: x0/x1 dispatch on the SP queue, weights+biases on the
            # ACT queue (emitted first so their dispatch overlaps the x DMA),
            # then the software pipeline:
            #   scores(k) | projections(k+1) | transposes(k+2) | tail(k)
            x0 = emit_x_dma(0, fine=True)
            x1 = emit_x_dma(1)
            load_weights()
            load_biases()
            xts = {0: conv_trans0(x0)}
            states = {0: proj(xts[0])}
            xts[1] = conv_trans(x1, dve_only=True)
            wns = {}
            for k in range(NCHUNK - 1):
                wns[k] = attn_scores(states[k][0], states[k][1])
                states[k + 1] = proj(xts[k + 1])
                if k + 2 < NCHUNK:
                    xts[k + 2] = load_trans(k + 2)
                ats = tail_watt(k, wns[k], states[k][2])
                if k < NCHUNK - 2:
                    tail_o(k, ats)
                else:
                    o_prev = tail_o(k, ats, js=[0, 1])
                    ats_prev = ats
            kl = NCHUNK - 1
            wns[kl] = attn_scores(states[kl][0], states[kl][1], act_norms=True)
            tail_o(kl - 1, ats_prev, js=[2, 3], o_sb=o_prev)
            ats = tail_watt(kl, wns[kl], states[kl][2])
            tail_o(kl, ats)

    nc.compile()
    return nc


def kernel(**inputs):
    if "nc" not in _CACHE:
        _CACHE["nc"] = build()
    nc = _CACHE["nc"]

    x = np.ascontiguousarray(np.asarray(inputs["x"], dtype=np.float32))
    shared = {
        "Wq16": np.ascontiguousarray(np.asarray(inputs["Wq"], dtype=np.float16)),
        "Wk16": np.ascontiguousarray(np.asarray(inputs["Wk"], dtype=np.float16)),
        "Wv16": np.ascontiguousarray(np.asarray(inputs["Wv"], dtype=np.float16)),
        "Wo16": np.ascontiguousarray(np.asarray(inputs["Wo"], dtype=np.float16)),
        "bqp": np.ascontiguousarray(
            np.asarray(inputs["bq"], dtype=np.float32).reshape(EC, 128).T
        ),
        "bkp": np.ascontiguousarray(
            np.asarray(inputs["bk"], dtype=np.float32).reshape(EC, 128).T
        ),
        "bv": np.ascontiguousarray(np.asarray(inputs["bv"], dtype=np.float32)),
        "bo": np.ascontiguousarray(np.asarray(inputs["bo"], dtype=np.float32)),
    }
    in_maps = [
        {"x": x[i * BLOC : (i + 1) * BLOC], **shared} for i in range(NCORES)
    ]
    res = run_bass_kernel_spmd(nc, in_maps, core_ids=list(range(NCORES)))
    return np.concatenate([res.results[i]["out"] for i in range(NCORES)], axis=0)
